# revision 36
# baseline (speedup 1.0000x reference)
"""Causal multi-head self-attention (RoPE) Trainium2 Bass kernel.

Problem: x:(4,2048,1024), Wq/Wk/Wv:(1024,1024), Wo:(1024,1024), bo:(1024,)
  q,k,v = split_heads(x@W*), rope(q), rope(k), causal softmax(q k^T/8) v, @Wo+bo

Sharding: head-parallel across 8 cores. Core c owns heads {2c, 2c+1} for all
4 batches: it computes q/k/v projections against the 128-column weight slice,
attention for its heads, and a partial output projection against the matching
128-row slice of Wo. Host sums the 8 partial (8192,1024) outputs and adds bo.

On-core layout (all "T" tensors are feature-major: partitions=feature rows,
free=tokens):
  Q^T/K^T (128 x 2048/batch): rows = [h0 d-evens(32), h0 d-odds(32), h1 ...]
    (NeoX-style d-permutation, folded into the host-permuted weight columns;
     valid because q and k get the same permutation and qk^T is d-invariant)
  RoPE: Q <- Q*cos + (P2@Q)*sin2, where P2 swaps the even/odd halves per head
    (PE matmul) and sin2 carries the sign; 3 DVE passes per tensor-block.
  S^T tiles (tj x ti) = K^T.T @ Q^T per head (fp32r, K=64 contraction).
  A = exp(0.125*S^T) (ACT, straddle tiles band-masked with -1e30 triangle).
  O~^T (65 x ti) accumulated = [V|1].T @ A over tj chunks; row 64 = softmax
    denominators (ones column trick). Normalize via ACT reciprocal +
    DRAM-staged partition broadcast + DVE multiply -> O^T (128 x 2048).
  y partial (128t x 1024) = O^T-chunk.T @ Wo-slice, DMA'd psum->DRAM.
"""

import numpy as np

B, T, C = 4, 2048, 1024
H, D = 16, 64
N_CORES = 8
BT = B * T
SCALE = 0.125  # D**-0.5
NEG = -1.0e30

TRACE = False            # set True (e.g. from test.py) to capture an NTFF trace
LAST_RESULT = None       # BassKernelResults of the most recent run

_BUILT = None            # cached (nc, input-name list)


# --------------------------------------------------------------------------
# workaround: this walrus build rejects >1 semaphore wait per instruction
def _split_sem_waits(nc, max_waits=1):
    import concourse.mybir as mybir

    n = 0
    for f in nc.m.functions:
        for bb in f.blocks:
            insts = bb.instructions
            idx = 0
            while idx < len(insts):
                i = insts[idx]
                si = getattr(i, "sync_info", None)
                if si is not None and si.on_wait and len(si.on_wait) > max_waits:
                    waits = list(si.on_wait)
                    extra, keep = waits[:-max_waits], waits[-max_waits:]
                    si.on_wait = keep
                    pos = idx
                    for j in range(0, len(extra), max_waits):
                        n += 1
                        nd = mybir.InstNoOp(name=f"I-waitsplit-{n}", ins=[], outs=[])
                        nd.engine = i.engine
                        nd.sync_info = mybir.SyncInfo(
                            on_wait=extra[j : j + max_waits], on_update=[]
                        )
                        insts.insert(pos, nd)
                        pos += 1
                    idx = pos
                idx += 1


def _install_ntff_hook():
    """The image's antenv lacks axon_hooks; synthesize it so trace=True works."""
    import sys
    import types

    if "antenv.axon_hooks" in sys.modules:
        return
    import antenv

    state = {"hook": None}
    mod = types.ModuleType("antenv.axon_hooks")
    mod.get_axon_ntff_profile_hook = lambda: state["hook"]
    mod.set_axon_ntff_profile_hook = lambda h: state.__setitem__("hook", h)
    sys.modules["antenv.axon_hooks"] = mod
    antenv.axon_hooks = mod
    try:
        from trn_agent_boot.trn_boot import _ntff_profile_via_ctypes

        state["hook"] = _ntff_profile_via_ctypes("/opt/axon/libaxon_pjrt.so")
    except Exception:
        state["hook"] = None


# --------------------------------------------------------------------------
def _build():
    import concourse.bass as bass
    import concourse.mybir as mybir
    from concourse.tile import TileContext

    F = mybir.dt.float32
    MD = mybir.dt.float16  # matmul operand dtype
    MULT = mybir.AluOpType.mult
    ADD = mybir.AluOpType.add
    SUB = mybir.AluOpType.subtract
    EXP = mybir.ActivationFunctionType.Exp

    nc = bass.Bass()

    xT = nc.dram_tensor("xT", (C, BT), MD, kind="ExternalInput")
    wq = nc.dram_tensor("wq", (C, 128), MD, kind="ExternalInput")
    wk = nc.dram_tensor("wk", (C, 128), MD, kind="ExternalInput")
    wv = nc.dram_tensor("wv", (C, 128), MD, kind="ExternalInput")
    wo = nc.dram_tensor("wo", (128, C), MD, kind="ExternalInput")
    cosd = nc.dram_tensor("cos", (128, T), MD, kind="ExternalInput")
    sind = nc.dram_tensor("sin2", (128, T), MD, kind="ExternalInput")
    p2d = nc.dram_tensor("p2", (128, 128), MD, kind="ExternalInput")
    bandd = nc.dram_tensor("band2x", (128, 256), F, kind="ExternalInput")
    id2d = nc.dram_tensor("id2", (128, 64), F, kind="ExternalInput")
    vonesd = nc.dram_tensor("vones", (128, 32), MD, kind="ExternalInput")
    vzerod = nc.dram_tensor("vzero", (128, 1008), MD, kind="ExternalInput")
    y = nc.dram_tensor("y", (BT, C), F, kind="ExternalOutput")
    scr_s = nc.dram_tensor("scr_s", (B * 8, 512), F, kind="Internal")
    scr = nc.dram_tensor("scr", (B * 8, 512), F, kind="Internal")

    with TileContext(nc) as tc:
        with (
            tc.tile_pool(name="const", bufs=1) as cst,
            tc.tile_pool(name="xt", bufs=3) as xtp,
            tc.tile_pool(name="qt", bufs=2) as qp,
            tc.tile_pool(name="kt", bufs=2) as kp,
            tc.tile_pool(name="vt", bufs=2) as vp,
            tc.tile_pool(name="ot", bufs=2) as op_,
            tc.tile_pool(name="vst", bufs=2) as vstp,
            tc.tile_pool(name="tmp", bufs=4) as tmp,
            tc.tile_pool(name="at", bufs=6) as ap_,
            tc.tile_pool(name="bc", bufs=4) as bcp,
            tc.tile_pool(name="avs", bufs=4) as avsp,
            tc.tile_pool(name="rr", bufs=4) as rp,
            tc.tile_pool(name="ys", bufs=4) as ysp,
            tc.tile_pool(name="sps", bufs=2, space="PSUM") as sps,
            tc.tile_pool(name="stp", bufs=2, space="PSUM") as stp,
            tc.tile_pool(name="avp", bufs=2, space="PSUM") as avp,
        ):
            # ---- constants -------------------------------------------------
            wq_t = cst.tile([128, 8, 128], MD)
            wk_t = cst.tile([128, 8, 128], MD)
            wv_t = cst.tile([128, 8, 128], MD)
            for k in range(8):
                nc.sync.dma_start(out=wq_t[:, k, :], in_=wq[k * 128 : (k + 1) * 128, :])
                nc.sync.dma_start(out=wk_t[:, k, :], in_=wk[k * 128 : (k + 1) * 128, :])
                nc.sync.dma_start(out=wv_t[:, k, :], in_=wv[k * 128 : (k + 1) * 128, :])
            wo_t = cst.tile([128, C], MD)
            nc.sync.dma_start(out=wo_t, in_=wo[:, :])
            cos_t = cst.tile([128, T], MD)
            nc.sync.dma_start(out=cos_t, in_=cosd[:, :])
            sin_t = cst.tile([128, T], MD)
            nc.sync.dma_start(out=sin_t, in_=sind[:, :])
            p2_t = cst.tile([128, 128], MD)
            nc.sync.dma_start(out=p2_t, in_=p2d[:, :])
            band_t = cst.tile([128, 256], F)  # [band | band] for head pairs
            nc.sync.dma_start(out=band_t, in_=bandd[:, :])
            id_t = cst.tile([128, 64], F)
            nc.sync.dma_start(out=id_t, in_=id2d[:, :])

            QKV = {}  # b -> (Qb, Kb, Vb);  O = {} b -> Ob

            def phase_a_alloc(b):
                Qb = qp.tile([128, T], MD, name="Qb")
                Kb = kp.tile([128, T], MD, name="Kb")
                Vb = vp.tile([128, 16, 256], MD, name="Vb")  # per head 128 cols:
                # [d 0..63 | ones | zeros*63] so the AV lhsT is 128-wide (FWL)
                QKV[b] = (Qb, Kb, Vb)
                nc.sync.dma_start(
                    out=Vb[:, :, 64:256:128],
                    in_=vonesd[:, :].rearrange("p (a b) -> p a b", b=2),
                )
                zin = vzerod[:, :].rearrange("p (a b) -> p a b", b=63)
                nc.sync.dma_start(out=Vb[:, :, 65:128], in_=zin)
                nc.sync.dma_start(out=Vb[:, :, 193:256], in_=zin)

            def phase_a_unit(b, nb):
                Qb, Kb, Vb = QKV[b]
                if True:
                    g0 = b * T + nb * 512
                    cols = slice(nb * 512, (nb + 1) * 512)
                    xt = xtp.tile([128, 8, 512], MD, name="xt")
                    for k in range(8):
                        nc.sync.dma_start(
                            out=xt[:, k, :],
                            in_=xT[k * 128 : (k + 1) * 128, g0 : g0 + 512],
                        )
                    for W, dst in ((wq_t, Qb), (wk_t, Kb)):
                        ps = sps.tile([128, 512], F, tag="s", name="ps")
                        for k in range(8):
                            nc.tensor.matmul(
                                ps[:, :], lhsT=W[:, k, :], rhs=xt[:, k, :],
                                start=(k == 0), stop=(k == 7),
                            )
                        # rope: dst = qr*cos - P2@(qr*sin2)
                        #   (P2@ (q.sin2))[p] = -q~[p]*sin2[p], since sin2 is
                        #    antisymmetric and cos symmetric under the pair swap
                        qr = tmp.tile([128, 512], MD, name="qr")
                        nc.scalar.copy(qr[:, :], ps[:, :])
                        qs = tmp.tile([128, 512], MD, name="qs")
                        nc.vector.tensor_tensor(qs[:, :], qr[:, :],
                                                sin_t[:, cols], MULT)
                        nc.vector.tensor_tensor(dst[:, cols], qr[:, :],
                                                cos_t[:, cols], MULT)
                        rot = sps.tile([128, 512], F, tag="s", name="rot")
                        nc.tensor.matmul(rot[:, :], lhsT=p2_t[:, :], rhs=qs[:, :],
                                         start=True, stop=True)
                        nc.vector.tensor_tensor(dst[:, cols], dst[:, cols],
                                                rot[:, :], SUB)
                    ps = sps.tile([128, 512], F, tag="s", name="ps")
                    for k in range(8):
                        nc.tensor.matmul(
                            ps[:, :], lhsT=wv_t[:, k, :], rhs=xt[:, k, :],
                            start=(k == 0), stop=(k == 7),
                        )
                    vst = vstp.tile([128, 512], F, name="vst")
                    nc.scalar.copy(vst[:, :], ps[:, :])
                    for tl in range(4):
                        tt = nb * 4 + tl
                        tcs = slice(tl * 128, (tl + 1) * 128)
                        for h in (0, 1):
                            tp = sps.tile([128, 64], F, tag="s", name="tp")
                            nc.tensor.transpose(
                                tp[:, :], vst[64 * h : 64 * h + 64, tcs],
                                id_t[64 * h : 64 * h + 64, :],
                            )
                            nc.vector.tensor_copy(
                                Vb[:, tt, 128 * h : 128 * h + 64], tp[:, :])

            def y_unit(b, Ob, i):
                # output projection for the 4 token-tiles of ti-block i
                for tt in range(4 * i, 4 * i + 4):
                    lhs = Ob[:, tt * 128 : (tt + 1) * 128]
                    ysb = ysp.tile([128, 1024], F, name="ysb")
                    for nh in (0, 1):
                        yps = sps.tile([128, 512], F, tag="s", name="yps")
                        nc.tensor.matmul(
                            yps[:, :], lhsT=lhs,
                            rhs=wo_t[:, nh * 512 : (nh + 1) * 512],
                            start=True, stop=True,
                        )
                        if nh == 0:
                            nc.vector.tensor_copy(ysb[:, 0:512], yps[:, :])
                        else:
                            nc.scalar.copy(ysb[:, 512:1024], yps[:, :])
                    r0 = b * T + tt * 128
                    nc.sync.dma_start(out=y[r0 : r0 + 128, :], in_=ysb[:, :])

            def phase_d(b, filler=None):
                Qb, Kb, Vb = QKV[b]
                Ob = op_.tile([128, T], MD, name="Ob")
                pending = []  # deferred y_units: keep normalize latency off
                # the PE critical path by emitting them a ti-block later
                for i in range(4):
                    av = [avp.tile([128, 512], F, tag="av", name="av")
                          for _ in (0, 1)]
                    nch = 4 * i + 4
                    sts = {}

                    def emit_st(j):
                        delta = j * 128 - i * 512
                        nl = 512 - max(0, delta)
                        off = 512 - nl
                        st = stp.tile([128, 2, 512], F, name="st")
                        for h in (0, 1):
                            hs = slice(64 * h, 64 * h + 64)
                            nc.tensor.matmul(
                                st[:, h, 0:nl],
                                lhsT=Kb[hs, j * 128 : (j + 1) * 128],
                                rhs=Qb[hs, i * 512 + off : (i + 1) * 512],
                                start=True, stop=True,
                            )
                        if delta >= 0:  # straddles the diagonal: mask triangle
                            nc.vector.tensor_tensor(
                                st[:, :, 0:128], st[:, :, 0:128],
                                band_t[:, :].rearrange("p (a c) -> p a c", a=2),
                                ADD)
                        sts[j] = (st, off, nl)

                    LAG = 1
                    for j in range(min(LAG, nch)):
                        emit_st(j)
                    for j in range(nch):
                        if j + LAG < nch:
                            emit_st(j + LAG)
                        if j == 1 and pending:
                            y_unit(b, Ob, pending.pop(0))
                        st, off, nl = sts.pop(j)
                        A = ap_.tile([128, 2, 512], MD, name="A")
                        nc.scalar.activation(
                            A[:, :, 0:nl], st[:, :, 0:nl], EXP, scale=SCALE)
                        for h in (0, 1):
                            nc.tensor.matmul(
                                av[h][0:128, off:512],
                                lhsT=Vb[:, j, 128 * h : 128 * h + 128],
                                rhs=A[:, h, 0:nl],
                                start=(j == 0), stop=(j == nch - 1),
                                skip_group_check=True,
                            )
                    for h in (0, 1):
                        row = b * 8 + i * 2 + h
                        # evacuate the accumulator to SBUF at once so the
                        # PSUM slot recycles without waiting on the
                        # reciprocal/broadcast DMA chain
                        avs = avsp.tile([65, 512], F, name="avs")
                        nc.vector.tensor_copy(avs[:, :], av[h][0:65, :])
                        # sums row -> DRAM -> (128x4) repartition -> lane-
                        # parallel reciprocal -> DRAM -> 64-row broadcast
                        srt = rp.tile([128, 4], F, name="srt")
                        nc.sync.dma_start(out=srt[:, :], in_=avs[64:65, :])
                        rt = rp.tile([128, 4], F, name="rt")
                        nc.vector.reciprocal(rt[:, :], srt[:, :])
                        nc.sync.dma_start(
                            out=scr[row : row + 1, :].rearrange(
                                "r (p c) -> (r p) c", c=4),
                            in_=rt[:, :],
                        )
                        bct = bcp.tile([64, 512], F, name="bct")
                        src = scr[row : row + 1, :]
                        bap = bass.AP(
                            tensor=src.tensor, offset=src.offset,
                            ap=[[0, 64]] + [list(p) for p in src.ap[1:]],
                        )
                        nc.sync.dma_start(out=bct[:, :], in_=bap)
                        nc.vector.tensor_tensor(
                            Ob[64 * h : 64 * h + 64, i * 512 : (i + 1) * 512],
                            avs[0:64, :], bct[:, :], MULT,
                        )
                    pending.append(i)
                    if filler is not None:
                        filler(i)
                for i2 in pending:
                    y_unit(b, Ob, i2)

            phase_a_alloc(0)
            for nb in range(4):
                phase_a_unit(0, nb)
            for b in range(B):
                if b + 1 < B:
                    phase_a_alloc(b + 1)
                    fil = (lambda i, nb=b + 1: phase_a_unit(nb, i))
                else:
                    fil = None
                phase_d(b, filler=fil)

    _split_sem_waits(nc)
    return nc


# --------------------------------------------------------------------------
def _host_inputs(x, Wq, Wk, Wv):
    """Per-core input dicts (all shared arrays built once)."""
    BF = np.float16
    xT = np.ascontiguousarray(
        np.asarray(x, dtype=np.float32).reshape(BT, C).T).astype(BF)

    # NeoX d-permutation within each head: evens then odds
    dperm = np.concatenate([np.arange(0, D, 2), np.arange(1, D, 2)])

    inv_freq = (1.0 / (10000.0 ** (np.arange(0, D, 2) / D))).astype(np.float64)
    pos = np.arange(T, dtype=np.float64)
    ang = pos[None, :] * inv_freq[:, None]  # (32, T)
    cos32 = np.cos(ang).astype(np.float32)
    sin32 = np.sin(ang).astype(np.float32)
    cos_t = np.tile(np.vstack([cos32, cos32]), (2, 1))  # (128, T)
    sin_t = np.tile(np.vstack([-sin32, sin32]), (2, 1))  # (128, T), sign folded

    p2 = np.zeros((128, 128), dtype=np.float32)
    for hb in (0, 64):
        for i2 in range(32):
            p2[hb + i2, hb + 32 + i2] = 1.0
            p2[hb + 32 + i2, hb + i2] = 1.0

    band = np.where(
        np.arange(128)[None, :] < np.arange(128)[:, None], np.float32(NEG), 0.0
    ).astype(np.float32)
    band2x = np.concatenate([band, band], axis=1)  # (128, 256)
    id2 = np.tile(np.eye(D, dtype=np.float32), (2, 1))  # (128, 64)

    Wq = np.asarray(Wq, dtype=np.float32)
    Wk = np.asarray(Wk, dtype=np.float32)
    Wv = np.asarray(Wv, dtype=np.float32)

    in_maps = []
    for c in range(N_CORES):
        sl = slice(128 * c, 128 * (c + 1))
        wq_c = Wq[:, sl].reshape(C, 2, D)[:, :, dperm].reshape(C, 128)
        wk_c = Wk[:, sl].reshape(C, 2, D)[:, :, dperm].reshape(C, 128)
        in_maps.append({
            "xT": xT,
            "wq": np.ascontiguousarray(wq_c).astype(BF),
            "wk": np.ascontiguousarray(wk_c).astype(BF),
            "wv": np.ascontiguousarray(Wv[:, sl]).astype(BF),
            "wo": None,  # set below
            "cos": cos_t.astype(BF),
            "sin2": sin_t.astype(BF),
            "p2": p2.astype(BF),
            "band2x": band2x,
            "id2": id2,
            "vones": np.ones((128, 32), dtype=BF),
            "vzero": np.zeros((128, 1008), dtype=BF),
        })
    return in_maps


def kernel(x, Wq, Wk, Wv, Wo, bo):
    global _BUILT, LAST_RESULT
    from concourse.bass_utils import run_bass_kernel_spmd

    if TRACE:
        _install_ntff_hook()

    if _BUILT is None:
        _BUILT = _build()
    nc = _BUILT

    in_maps = _host_inputs(x, Wq, Wk, Wv)
    Wo = np.asarray(Wo, dtype=np.float32)
    for c in range(N_CORES):
        in_maps[c]["wo"] = np.ascontiguousarray(
            Wo[128 * c : 128 * (c + 1), :]).astype(np.float16)

    res = run_bass_kernel_spmd(
        nc, in_maps, core_ids=list(range(N_CORES)), trace=TRACE
    )
    LAST_RESULT = res

    acc = res.results[0]["y"].astype(np.float64)
    for c in range(1, N_CORES):
        acc = acc + res.results[c]["y"]
    out = acc.astype(np.float32) + np.asarray(bo, dtype=np.float32)[None, :]
    return out.reshape(B, T, C)


# revision 37
# speedup vs baseline: 1.0347x; 1.0347x over previous
"""Causal multi-head self-attention (RoPE) Trainium2 Bass kernel.

Problem: x:(4,2048,1024), Wq/Wk/Wv:(1024,1024), Wo:(1024,1024), bo:(1024,)
  q,k,v = split_heads(x@W*), rope(q), rope(k), causal softmax(q k^T/8) v, @Wo+bo

Sharding: head-parallel across 8 cores. Core c owns heads {2c, 2c+1} for all
4 batches: it computes q/k/v projections against the 128-column weight slice,
attention for its heads, and a partial output projection against the matching
128-row slice of Wo. Host sums the 8 partial (8192,1024) outputs and adds bo.

On-core layout (all "T" tensors are feature-major: partitions=feature rows,
free=tokens):
  Q^T/K^T (128 x 2048/batch): rows = [h0 d-evens(32), h0 d-odds(32), h1 ...]
    (NeoX-style d-permutation, folded into the host-permuted weight columns;
     valid because q and k get the same permutation and qk^T is d-invariant)
  RoPE: Q <- Q*cos + (P2@Q)*sin2, where P2 swaps the even/odd halves per head
    (PE matmul) and sin2 carries the sign; 3 DVE passes per tensor-block.
  S^T tiles (tj x ti) = K^T.T @ Q^T per head (fp32r, K=64 contraction).
  A = exp(0.125*S^T) (ACT, straddle tiles band-masked with -1e30 triangle).
  O~^T (65 x ti) accumulated = [V|1].T @ A over tj chunks; row 64 = softmax
    denominators (ones column trick). Normalize via ACT reciprocal +
    DRAM-staged partition broadcast + DVE multiply -> O^T (128 x 2048).
  y partial (128t x 1024) = O^T-chunk.T @ Wo-slice, DMA'd psum->DRAM.
"""

import numpy as np

B, T, C = 4, 2048, 1024
H, D = 16, 64
N_CORES = 8
BT = B * T
SCALE = 0.125  # D**-0.5
NEG = -1.0e30

TRACE = False            # set True (e.g. from test.py) to capture an NTFF trace
LAST_RESULT = None       # BassKernelResults of the most recent run

_BUILT = None            # cached (nc, input-name list)


# --------------------------------------------------------------------------
# workaround: this walrus build rejects >1 semaphore wait per instruction
def _split_sem_waits(nc, max_waits=1):
    import concourse.mybir as mybir

    n = 0
    for f in nc.m.functions:
        for bb in f.blocks:
            insts = bb.instructions
            idx = 0
            while idx < len(insts):
                i = insts[idx]
                si = getattr(i, "sync_info", None)
                if si is not None and si.on_wait and len(si.on_wait) > max_waits:
                    waits = list(si.on_wait)
                    extra, keep = waits[:-max_waits], waits[-max_waits:]
                    si.on_wait = keep
                    pos = idx
                    for j in range(0, len(extra), max_waits):
                        n += 1
                        nd = mybir.InstNoOp(name=f"I-waitsplit-{n}", ins=[], outs=[])
                        nd.engine = i.engine
                        nd.sync_info = mybir.SyncInfo(
                            on_wait=extra[j : j + max_waits], on_update=[]
                        )
                        insts.insert(pos, nd)
                        pos += 1
                    idx = pos
                idx += 1


def _install_ntff_hook():
    """The image's antenv lacks axon_hooks; synthesize it so trace=True works."""
    import sys
    import types

    if "antenv.axon_hooks" in sys.modules:
        return
    import antenv

    state = {"hook": None}
    mod = types.ModuleType("antenv.axon_hooks")
    mod.get_axon_ntff_profile_hook = lambda: state["hook"]
    mod.set_axon_ntff_profile_hook = lambda h: state.__setitem__("hook", h)
    sys.modules["antenv.axon_hooks"] = mod
    antenv.axon_hooks = mod
    try:
        from trn_agent_boot.trn_boot import _ntff_profile_via_ctypes

        state["hook"] = _ntff_profile_via_ctypes("/opt/axon/libaxon_pjrt.so")
    except Exception:
        state["hook"] = None


# --------------------------------------------------------------------------
def _build():
    import concourse.bass as bass
    import concourse.mybir as mybir
    from concourse.tile import TileContext

    F = mybir.dt.float32
    MD = mybir.dt.float16  # matmul operand dtype
    MULT = mybir.AluOpType.mult
    ADD = mybir.AluOpType.add
    SUB = mybir.AluOpType.subtract
    EXP = mybir.ActivationFunctionType.Exp

    nc = bass.Bass()

    xT = nc.dram_tensor("xT", (C, BT), MD, kind="ExternalInput")
    wq = nc.dram_tensor("wq", (C, 128), MD, kind="ExternalInput")
    wk = nc.dram_tensor("wk", (C, 128), MD, kind="ExternalInput")
    wv = nc.dram_tensor("wv", (C, 128), MD, kind="ExternalInput")
    wo = nc.dram_tensor("wo", (128, C), MD, kind="ExternalInput")
    cosd = nc.dram_tensor("cos", (128, T), MD, kind="ExternalInput")
    sind = nc.dram_tensor("sin2", (128, T), MD, kind="ExternalInput")
    p2d = nc.dram_tensor("p2", (128, 128), MD, kind="ExternalInput")
    bandd = nc.dram_tensor("band2x", (128, 256), F, kind="ExternalInput")
    id2d = nc.dram_tensor("id2", (128, 64), F, kind="ExternalInput")
    vonesd = nc.dram_tensor("vones", (128, 32), MD, kind="ExternalInput")
    vzerod = nc.dram_tensor("vzero", (128, 1008), MD, kind="ExternalInput")
    y = nc.dram_tensor("y", (BT, C), F, kind="ExternalOutput")
    scr_s = nc.dram_tensor("scr_s", (B * 8, 512), F, kind="Internal")
    scr = nc.dram_tensor("scr", (B * 8, 512), F, kind="Internal")

    with TileContext(nc) as tc:
        with (
            tc.tile_pool(name="const", bufs=1) as cst,
            tc.tile_pool(name="xt", bufs=3) as xtp,
            tc.tile_pool(name="qt", bufs=2) as qp,
            tc.tile_pool(name="kt", bufs=2) as kp,
            tc.tile_pool(name="vt", bufs=2) as vp,
            tc.tile_pool(name="ot", bufs=2) as op_,
            tc.tile_pool(name="vst", bufs=2) as vstp,
            tc.tile_pool(name="tmp", bufs=4) as tmp,
            tc.tile_pool(name="at", bufs=6) as ap_,
            tc.tile_pool(name="bc", bufs=4) as bcp,
            tc.tile_pool(name="avs", bufs=4) as avsp,
            tc.tile_pool(name="rr", bufs=4) as rp,
            tc.tile_pool(name="ys", bufs=4) as ysp,
            tc.tile_pool(name="sps", bufs=2, space="PSUM") as sps,
            tc.tile_pool(name="stp", bufs=2, space="PSUM") as stp,
            tc.tile_pool(name="avp", bufs=2, space="PSUM") as avp,
        ):
            # ---- constants -------------------------------------------------
            wq_t = cst.tile([128, 8, 128], MD)
            wk_t = cst.tile([128, 8, 128], MD)
            wv_t = cst.tile([128, 8, 128], MD)
            for k in range(8):
                nc.sync.dma_start(out=wq_t[:, k, :], in_=wq[k * 128 : (k + 1) * 128, :])
                nc.sync.dma_start(out=wk_t[:, k, :], in_=wk[k * 128 : (k + 1) * 128, :])
                nc.sync.dma_start(out=wv_t[:, k, :], in_=wv[k * 128 : (k + 1) * 128, :])
            wo_t = cst.tile([128, C], MD)
            nc.sync.dma_start(out=wo_t, in_=wo[:, :])
            cos_t = cst.tile([128, T], MD)
            nc.sync.dma_start(out=cos_t, in_=cosd[:, :])
            sin_t = cst.tile([128, T], MD)
            nc.sync.dma_start(out=sin_t, in_=sind[:, :])
            p2_t = cst.tile([128, 128], MD)
            nc.sync.dma_start(out=p2_t, in_=p2d[:, :])
            band_t = cst.tile([128, 256], F)  # [band | band] for head pairs
            nc.sync.dma_start(out=band_t, in_=bandd[:, :])
            id_t = cst.tile([128, 64], F)
            nc.sync.dma_start(out=id_t, in_=id2d[:, :])

            QKV = {}  # b -> (Qb, Kb, Vb);  O = {} b -> Ob

            def phase_a_alloc(b):
                Qb = qp.tile([128, T], MD, name="Qb")
                Kb = kp.tile([128, T], MD, name="Kb")
                Vb = vp.tile([128, 16, 256], MD, name="Vb")  # per head 128 cols:
                # [d 0..63 | ones | zeros*63] so the AV lhsT is 128-wide (FWL)
                QKV[b] = (Qb, Kb, Vb)
                nc.sync.dma_start(
                    out=Vb[:, :, 64:256:128],
                    in_=vonesd[:, :].rearrange("p (a b) -> p a b", b=2),
                )
                zin = vzerod[:, :].rearrange("p (a b) -> p a b", b=63)
                nc.sync.dma_start(out=Vb[:, :, 65:128], in_=zin)
                nc.sync.dma_start(out=Vb[:, :, 193:256], in_=zin)

            def phase_a_unit(b, nb):
                Qb, Kb, Vb = QKV[b]
                if True:
                    g0 = b * T + nb * 512
                    cols = slice(nb * 512, (nb + 1) * 512)
                    xt = xtp.tile([128, 8, 512], MD, name="xt")
                    for k in range(8):
                        nc.sync.dma_start(
                            out=xt[:, k, :],
                            in_=xT[k * 128 : (k + 1) * 128, g0 : g0 + 512],
                        )
                    for W, dst in ((wq_t, Qb), (wk_t, Kb)):
                        ps = sps.tile([128, 512], F, tag="s", name="ps")
                        for k in range(8):
                            nc.tensor.matmul(
                                ps[:, :], lhsT=W[:, k, :], rhs=xt[:, k, :],
                                start=(k == 0), stop=(k == 7),
                            )
                        # rope: dst = qr*cos - P2@(qr*sin2)
                        #   (P2@ (q.sin2))[p] = -q~[p]*sin2[p], since sin2 is
                        #    antisymmetric and cos symmetric under the pair swap
                        qr = tmp.tile([128, 512], MD, name="qr")
                        nc.scalar.copy(qr[:, :], ps[:, :])
                        qs = tmp.tile([128, 512], MD, name="qs")
                        nc.vector.tensor_tensor(qs[:, :], qr[:, :],
                                                sin_t[:, cols], MULT)
                        nc.vector.tensor_tensor(dst[:, cols], qr[:, :],
                                                cos_t[:, cols], MULT)
                        rot = sps.tile([128, 512], F, tag="s", name="rot")
                        nc.tensor.matmul(rot[:, :], lhsT=p2_t[:, :], rhs=qs[:, :],
                                         start=True, stop=True)
                        nc.vector.tensor_tensor(dst[:, cols], dst[:, cols],
                                                rot[:, :], SUB)
                    ps = sps.tile([128, 512], F, tag="s", name="ps")
                    for k in range(8):
                        nc.tensor.matmul(
                            ps[:, :], lhsT=wv_t[:, k, :], rhs=xt[:, k, :],
                            start=(k == 0), stop=(k == 7),
                        )
                    vst = vstp.tile([128, 512], F, name="vst")
                    nc.scalar.copy(vst[:, :], ps[:, :])
                    for tl in range(4):
                        tt = nb * 4 + tl
                        tcs = slice(tl * 128, (tl + 1) * 128)
                        for h in (0, 1):
                            tp = sps.tile([128, 64], F, tag="s", name="tp")
                            nc.tensor.transpose(
                                tp[:, :], vst[64 * h : 64 * h + 64, tcs],
                                id_t[64 * h : 64 * h + 64, :],
                            )
                            nc.vector.tensor_copy(
                                Vb[:, tt, 128 * h : 128 * h + 64], tp[:, :])

            def y_unit(b, Ob, i):
                # output projection for the 4 token-tiles of ti-block i
                for tt in range(4 * i, 4 * i + 4):
                    lhs = Ob[:, tt * 128 : (tt + 1) * 128]
                    ysb = ysp.tile([128, 1024], F, name="ysb")
                    for nh in (0, 1):
                        yps = sps.tile([128, 512], F, tag="s", name="yps")
                        nc.tensor.matmul(
                            yps[:, :], lhsT=lhs,
                            rhs=wo_t[:, nh * 512 : (nh + 1) * 512],
                            start=True, stop=True,
                        )
                        if nh == 0:
                            nc.vector.tensor_copy(ysb[:, 0:512], yps[:, :])
                        else:
                            nc.scalar.copy(ysb[:, 512:1024], yps[:, :])
                    r0 = b * T + tt * 128
                    nc.sync.dma_start(out=y[r0 : r0 + 128, :], in_=ysb[:, :])

            def phase_d(b, filler=None, pre=None):
                Qb, Kb, Vb = QKV[b]
                Ob = op_.tile([128, T], MD, name="Ob")
                pending = []  # deferred y_units: keep normalize latency off
                # the PE critical path by emitting them a ti-block later
                for i in range(4):
                    if pre is not None:
                        pre(i)
                    av = [avp.tile([128, 512], F, tag="av", name="av")
                          for _ in (0, 1)]
                    nch = 4 * i + 4
                    sts = {}

                    def emit_st(j):
                        delta = j * 128 - i * 512
                        nl = 512 - max(0, delta)
                        off = 512 - nl
                        st = stp.tile([128, 2, 512], F, name="st")
                        for h in (0, 1):
                            hs = slice(64 * h, 64 * h + 64)
                            nc.tensor.matmul(
                                st[:, h, 0:nl],
                                lhsT=Kb[hs, j * 128 : (j + 1) * 128],
                                rhs=Qb[hs, i * 512 + off : (i + 1) * 512],
                                start=True, stop=True,
                            )
                        if delta >= 0:  # straddles the diagonal: mask triangle
                            nc.vector.tensor_tensor(
                                st[:, :, 0:128], st[:, :, 0:128],
                                band_t[:, :].rearrange("p (a c) -> p a c", a=2),
                                ADD)
                        sts[j] = (st, off, nl)

                    LAG = 1
                    for j in range(min(LAG, nch)):
                        emit_st(j)
                    for j in range(nch):
                        if j + LAG < nch:
                            emit_st(j + LAG)
                        if j == 1 and pending:
                            y_unit(b, Ob, pending.pop(0))
                        st, off, nl = sts.pop(j)
                        A = ap_.tile([128, 2, 512], MD, name="A")
                        nc.scalar.activation(
                            A[:, :, 0:nl], st[:, :, 0:nl], EXP, scale=SCALE)
                        for h in (0, 1):
                            nc.tensor.matmul(
                                av[h][0:128, off:512],
                                lhsT=Vb[:, j, 128 * h : 128 * h + 128],
                                rhs=A[:, h, 0:nl],
                                start=(j == 0), stop=(j == nch - 1),
                                skip_group_check=True,
                            )
                    for h in (0, 1):
                        row = b * 8 + i * 2 + h
                        # evacuate the accumulator to SBUF at once so the
                        # PSUM slot recycles without waiting on the
                        # reciprocal/broadcast DMA chain
                        avs = avsp.tile([65, 512], F, name="avs")
                        nc.vector.tensor_copy(avs[:, :], av[h][0:65, :])
                        # sums row -> DRAM -> (128x4) repartition -> lane-
                        # parallel reciprocal -> DRAM -> 64-row broadcast
                        srt = rp.tile([128, 4], F, name="srt")
                        nc.sync.dma_start(out=srt[:, :], in_=avs[64:65, :])
                        rt = rp.tile([128, 4], F, name="rt")
                        nc.vector.reciprocal(rt[:, :], srt[:, :])
                        nc.sync.dma_start(
                            out=scr[row : row + 1, :].rearrange(
                                "r (p c) -> (r p) c", c=4),
                            in_=rt[:, :],
                        )
                        bct = bcp.tile([64, 512], F, name="bct")
                        src = scr[row : row + 1, :]
                        bap = bass.AP(
                            tensor=src.tensor, offset=src.offset,
                            ap=[[0, 64]] + [list(p) for p in src.ap[1:]],
                        )
                        nc.sync.dma_start(out=bct[:, :], in_=bap)
                        nc.vector.tensor_tensor(
                            Ob[64 * h : 64 * h + 64, i * 512 : (i + 1) * 512],
                            avs[0:64, :], bct[:, :], MULT,
                        )
                    pending.append(i)
                    if filler is not None:
                        filler(i)
                for i2 in pending:
                    y_unit(b, Ob, i2)

            phase_a_alloc(0)
            for b in range(B):
                if b + 1 < B:
                    phase_a_alloc(b + 1)
                    fil = (lambda i, nb=b + 1: phase_a_unit(nb, i))
                else:
                    fil = None
                # batch 0's projection blocks are emitted just-in-time ahead
                # of the attention block that first needs them
                pre = (lambda i: phase_a_unit(0, i)) if b == 0 else None
                phase_d(b, filler=fil, pre=pre)

    _split_sem_waits(nc)
    return nc


# --------------------------------------------------------------------------
def _host_inputs(x, Wq, Wk, Wv):
    """Per-core input dicts (all shared arrays built once)."""
    BF = np.float16
    xT = np.ascontiguousarray(
        np.asarray(x, dtype=np.float32).reshape(BT, C).T).astype(BF)

    # NeoX d-permutation within each head: evens then odds
    dperm = np.concatenate([np.arange(0, D, 2), np.arange(1, D, 2)])

    inv_freq = (1.0 / (10000.0 ** (np.arange(0, D, 2) / D))).astype(np.float64)
    pos = np.arange(T, dtype=np.float64)
    ang = pos[None, :] * inv_freq[:, None]  # (32, T)
    cos32 = np.cos(ang).astype(np.float32)
    sin32 = np.sin(ang).astype(np.float32)
    cos_t = np.tile(np.vstack([cos32, cos32]), (2, 1))  # (128, T)
    sin_t = np.tile(np.vstack([-sin32, sin32]), (2, 1))  # (128, T), sign folded

    p2 = np.zeros((128, 128), dtype=np.float32)
    for hb in (0, 64):
        for i2 in range(32):
            p2[hb + i2, hb + 32 + i2] = 1.0
            p2[hb + 32 + i2, hb + i2] = 1.0

    band = np.where(
        np.arange(128)[None, :] < np.arange(128)[:, None], np.float32(NEG), 0.0
    ).astype(np.float32)
    band2x = np.concatenate([band, band], axis=1)  # (128, 256)
    id2 = np.tile(np.eye(D, dtype=np.float32), (2, 1))  # (128, 64)

    Wq = np.asarray(Wq, dtype=np.float32)
    Wk = np.asarray(Wk, dtype=np.float32)
    Wv = np.asarray(Wv, dtype=np.float32)

    in_maps = []
    for c in range(N_CORES):
        sl = slice(128 * c, 128 * (c + 1))
        wq_c = Wq[:, sl].reshape(C, 2, D)[:, :, dperm].reshape(C, 128)
        wk_c = Wk[:, sl].reshape(C, 2, D)[:, :, dperm].reshape(C, 128)
        in_maps.append({
            "xT": xT,
            "wq": np.ascontiguousarray(wq_c).astype(BF),
            "wk": np.ascontiguousarray(wk_c).astype(BF),
            "wv": np.ascontiguousarray(Wv[:, sl]).astype(BF),
            "wo": None,  # set below
            "cos": cos_t.astype(BF),
            "sin2": sin_t.astype(BF),
            "p2": p2.astype(BF),
            "band2x": band2x,
            "id2": id2,
            "vones": np.ones((128, 32), dtype=BF),
            "vzero": np.zeros((128, 1008), dtype=BF),
        })
    return in_maps


def kernel(x, Wq, Wk, Wv, Wo, bo):
    global _BUILT, LAST_RESULT
    from concourse.bass_utils import run_bass_kernel_spmd

    if TRACE:
        _install_ntff_hook()

    if _BUILT is None:
        _BUILT = _build()
    nc = _BUILT

    in_maps = _host_inputs(x, Wq, Wk, Wv)
    Wo = np.asarray(Wo, dtype=np.float32)
    for c in range(N_CORES):
        in_maps[c]["wo"] = np.ascontiguousarray(
            Wo[128 * c : 128 * (c + 1), :]).astype(np.float16)

    res = run_bass_kernel_spmd(
        nc, in_maps, core_ids=list(range(N_CORES)), trace=TRACE
    )
    LAST_RESULT = res

    acc = res.results[0]["y"].astype(np.float64)
    for c in range(1, N_CORES):
        acc = acc + res.results[c]["y"]
    out = acc.astype(np.float32) + np.asarray(bo, dtype=np.float32)[None, :]
    return out.reshape(B, T, C)
